# revision 1
# baseline (speedup 1.0000x reference)
"""DetectionIOUMetric Trainium2 kernel.

Computes, for pred_boxes [32, 4096, 6] and gt_boxes [32, 1024, 6] (cx, cy, w, h
in the first 4 channels; a box is padding iff cx == -1):

    masked pairwise IoU, num_pos / num_true / num_pred / num_gt per batch,
    precision / recall / F1 per batch.

Sharding: pure data parallel over the batch dim — each of the 8 NeuronCores
processes 4 batches; no cross-device communication. The device program
computes the four integer counts per batch; the trivial final eps-divisions
are applied on the host after the gather.

Device algorithm per batch (fp32), gt-on-partitions layout:
  iou > 0.5  <=>  3*inter - area_g > area_p + eps   (union+eps > 0)
  inter = relu(wx) * wy  (one-sided relu suffices: wy<0 makes the product
  non-positive, which always fails the strict > test).
  gt boxes live on partitions (8 chunks of 128), preds on the free dim
  (FD=4096, pred-side quantities broadcast to all partitions):
    vx     = min(-px1_t, -gx1)                    tensor_scalar   (GpSimd)
    wx     = min(px2_t, gx2) + vx                 scalar_tensor_tensor (DVE)
    vy     = min(-py1_t, -gy1)                    tensor_scalar   (GpSimd)
    wy     = min(py2_t, gy2) + vy                 scalar_tensor_tensor (DVE)
    wxr3   = relu(3*wx)                           activation      (ACT)
    inter3 = wxr3 * wy                            tensor_tensor   (GpSimd+DVE)
    condv  = (inter3 - ag) > apeps_t              scalar_tensor_tensor (DVE)
             + accum_out = per-gt match count S  -> num_true
  PE accumulates per-pred column sums of condv over the 8 gt chunks
  -> num_pos = count(colsum > 0).
  Pred-side rows are derived in an [8, 3072] layout, staged to DRAM in pred
  order, and broadcast to [128, 5*4096] with a log-doubling DMA chain
  (large contiguous runs; step-0 broadcast APs degenerate to per-element
  DMA descriptors and must be avoided).
"""
import os
import numpy as np

import concourse.bass as bass
import concourse.bacc as bacc
import concourse.tile as tile
from concourse import mybir
from concourse.bass_utils import run_bass_kernel_spmd

F32 = mybir.dt.float32
EPS = 1e-7
IOU_PENALTY = 1e30

B_TOTAL = 32
N_CORES = 8
REPEAT = 1                     # timing-calibration knob (outputs idempotent)
BPC = B_TOTAL // N_CORES       # batches per core
P = 4096                       # pred boxes per batch (free dim)
G = 1024                       # gt boxes per batch (8 partition chunks)
NCH = G // 128                 # 8 gt chunks per batch
MSPLIT = 2560                  # inter3 columns on GpSimd; rest on DVE

_PROGRAM_CACHE = {}

Alu = mybir.AluOpType


def _build(with_mask: bool, repeat: int = None):
    """One SPMD program: inputs pred [BPC, P, 6] / gt [BPC, G, 6],
    output counts [1, 16] = per-batch [num_pos, num_pred, num_gt, num_true]."""
    if repeat is None:
        repeat = REPEAT
    NROW = 6 if with_mask else 5
    nc = bacc.Bacc(None, target_bir_lowering=False)
    pred_d = nc.dram_tensor("pred", [BPC, P, 6], F32, kind="ExternalInput")
    gt_d = nc.dram_tensor("gt", [BPC, G, 6], F32, kind="ExternalInput")
    counts_d = nc.dram_tensor("counts", [1, 16], F32, kind="ExternalOutput")

    with tile.TileContext(nc) as tc:
        with (
            tc.tile_pool(name="cst", bufs=1) as cst,
            tc.tile_pool(name="rows", bufs=2) as rows,
            tc.tile_pool(name="gtp", bufs=1) as gtp,
            tc.tile_pool(name="sca", bufs=2) as sca,
            tc.tile_pool(name="wk", bufs=1) as wk,
            tc.tile_pool(name="ps", bufs=1, space=bass.MemorySpace.PSUM) as ps,
            tc.tile_pool(name="dram", bufs=2, space=bass.MemorySpace.DRAM) as dram,
        ):
            ones128 = cst.tile([128, 1], F32)
            nc.vector.memset(ones128[:], 1.0)
            counts_sb = cst.tile([128, 16], F32)
            nc.vector.memset(counts_sb[:], 0.0)

            for b in [bb for _ in range(repeat) for bb in range(BPC)]:
                # ---------- pred prep: derive rows, stage, broadcast ----------
                # [32, 768]: partition q holds pred boxes 128q .. 128q+127
                pred_lin = rows.tile([32, 768], F32)
                nc.sync.dma_start(
                    pred_lin[:],
                    pred_d.ap()[b].rearrange("(q x) c -> q (x c)", q=32),
                )
                r3p = pred_lin[:].rearrange("q (x c) -> q x c", c=6)
                pcx = r3p[:, :, 0]
                pcy = r3p[:, :, 1]
                pw = r3p[:, :, 2]
                ph = r3p[:, :, 3]
                psmall = rows.tile([32, NROW * 128], F32)
                px2_s = psmall[:, 0:128]
                mpx1_s = psmall[:, 128:256]
                py2_s = psmall[:, 256:384]
                mpy1_s = psmall[:, 384:512]
                apeps_s = psmall[:, 512:640]
                nc.vector.scalar_tensor_tensor(
                    px2_s, pw, 0.5, pcx, op0=Alu.mult, op1=Alu.add)
                nc.vector.scalar_tensor_tensor(
                    mpx1_s, pw, 0.5, pcx, op0=Alu.mult, op1=Alu.subtract)
                nc.vector.scalar_tensor_tensor(
                    py2_s, ph, 0.5, pcy, op0=Alu.mult, op1=Alu.add)
                nc.vector.scalar_tensor_tensor(
                    mpy1_s, ph, 0.5, pcy, op0=Alu.mult, op1=Alu.subtract)
                # area exactly as the reference: (px2-px1)*(py2-py1), +eps
                dx_s = sca.tile([32, 128], F32, tag="dx_s", name="dx_s")
                dy_s = sca.tile([32, 128], F32, tag="dy_s", name="dy_s")
                nc.vector.tensor_tensor(dx_s[:], px2_s, mpx1_s, op=Alu.add)
                nc.vector.tensor_tensor(dy_s[:], py2_s, mpy1_s, op=Alu.add)
                nc.vector.tensor_tensor(apeps_s, dx_s[:], dy_s[:], op=Alu.mult)
                nc.vector.tensor_scalar(
                    apeps_s, apeps_s, EPS, None, op0=Alu.add)
                if with_mask:
                    nc.vector.tensor_scalar(
                        psmall[:, 640:768], pcx, -1.0, None, op0=Alu.is_equal)

                if with_mask:
                    # pred validity -> counts_sb[0:32, 4+b]
                    vp = sca.tile([32, 128], F32, tag="vp", name="vp")
                    nc.vector.tensor_scalar(
                        vp[:], pcx, -1.0, None, op0=Alu.not_equal)
                    nc.vector.tensor_reduce(
                        counts_sb[0:32, 4 + b : 5 + b], vp[:],
                        axis=mybir.AxisListType.X, op=Alu.add)

                # stage to DRAM in pred order: scr[t, 128q+j] = psmall[q, 128t+j]
                scr = dram.tile([NROW, P], F32)
                nc.sync.dma_start(
                    scr[:].rearrange("t (q j) -> q t j", j=128),
                    psmall[:].rearrange("q (t j) -> q t j", j=128),
                )
                # broadcast: big[p, t*P + i] = row t, pred i, for all p.
                # 8 partition-group DMAs, each re-reading the scratch row with
                # an outer step-0 rep dim (inner runs stay 20KB-contiguous).
                big = gtp.tile([128, NROW * P], F32, tag="big", name="big")
                scr_flat = scr[:].rearrange("t g -> (t g)")
                H = NROW * P // 2
                for g4 in range(4):
                    for h2 in range(2):
                        nc.sync.dma_start(
                            big[g4 * 32 : (g4 + 1) * 32,
                                h2 * H : (h2 + 1) * H],
                            scr_flat[None, None, h2 * H : (h2 + 1) * H]
                            .broadcast_to([1, 32, H]),
                        )
                px2_t = big[:, 0 * P : 1 * P]
                mpx1_t = big[:, 1 * P : 2 * P]
                py2_t = big[:, 2 * P : 3 * P]
                mpy1_t = big[:, 3 * P : 4 * P]
                apeps_t = big[:, 4 * P : 5 * P]
                if with_mask:
                    invp_t = big[:, 5 * P : 6 * P]

                # ---------- gt prep: per-chunk scalars ----------
                # [128, 48]: partition p holds gt boxes 8p .. 8p+7;
                # chunk j pairs partition p with gt box 8p+j (order-invariant)
                gt_lin = rows.tile([128, 48], F32)
                nc.sync.dma_start(
                    gt_lin[:], gt_d.ap()[b].rearrange("(q x) c -> q (x c)", q=128)
                )
                r3g = gt_lin[:].rearrange("q (x c) -> q x c", c=6)
                gcx = r3g[:, :, 0]
                gcy = r3g[:, :, 1]
                gw = r3g[:, :, 2]
                gh = r3g[:, :, 3]
                gscal = sca.tile([128, 48], F32, tag="gscal", name="gscal")
                gx2_c = gscal[:, 0:8]
                mgx1_c = gscal[:, 8:16]
                gy2_c = gscal[:, 16:24]
                mgy1_c = gscal[:, 24:32]
                ag_c = gscal[:, 32:40]
                nc.vector.scalar_tensor_tensor(
                    gx2_c, gw, 0.5, gcx, op0=Alu.mult, op1=Alu.add)
                nc.vector.scalar_tensor_tensor(
                    mgx1_c, gw, 0.5, gcx, op0=Alu.mult, op1=Alu.subtract)
                nc.vector.scalar_tensor_tensor(
                    gy2_c, gh, 0.5, gcy, op0=Alu.mult, op1=Alu.add)
                nc.vector.scalar_tensor_tensor(
                    mgy1_c, gh, 0.5, gcy, op0=Alu.mult, op1=Alu.subtract)
                nc.vector.tensor_tensor(ag_c, gw, gh, op=Alu.mult)
                if with_mask:
                    nc.vector.tensor_scalar(
                        gscal[:, 40:48], gcx, -1.0, IOU_PENALTY,
                        op0=Alu.is_equal, op1=Alu.mult)

                if with_mask:
                    # gt validity -> counts_sb[:, 8+b]
                    vg = sca.tile([128, 8], F32, tag="vg", name="vg")
                    nc.vector.tensor_scalar(
                        vg[:], gcx, -1.0, None, op0=Alu.not_equal)
                    nc.vector.tensor_reduce(
                        counts_sb[:, 8 + b : 9 + b], vg[:],
                        axis=mybir.AxisListType.X, op=Alu.add)

                # ---------- chunk loop over 8 gt chunks ----------
                Scol = sca.tile([128, NCH], F32, tag="Scol", name="Scol")
                nt = ps.tile([1, P], F32, tag="nt", name="nt")
                for c in range(NCH):
                    vx = wk.tile([128, P], F32, tag="vx", name="vx")
                    nc.gpsimd.tensor_scalar(
                        vx[:], mpx1_t, mgx1_c[:, c : c + 1], None, op0=Alu.min)
                    wx = wk.tile([128, P], F32, tag="wx", name="wx")
                    nc.vector.scalar_tensor_tensor(
                        wx[:], px2_t, gx2_c[:, c : c + 1], vx[:],
                        op0=Alu.min, op1=Alu.add)
                    vy = wk.tile([128, P], F32, tag="vy", name="vy")
                    nc.gpsimd.tensor_scalar(
                        vy[:], mpy1_t, mgy1_c[:, c : c + 1], None, op0=Alu.min)
                    wy = wk.tile([128, P], F32, tag="wy", name="wy")
                    nc.vector.scalar_tensor_tensor(
                        wy[:], py2_t, gy2_c[:, c : c + 1], vy[:],
                        op0=Alu.min, op1=Alu.add)
                    # wxr3 reuses vx's slot, inter3 reuses vy's slot,
                    # condv reuses wx's slot (SBUF pressure)
                    wxr3 = wk.tile([128, P], F32, tag="vx", name="wxr3")
                    nc.scalar.activation(
                        wxr3[:], wx[:], mybir.ActivationFunctionType.Relu,
                        scale=3.0)
                    inter3 = wk.tile([128, P], F32, tag="vy", name="inter3")
                    nc.gpsimd.tensor_tensor(
                        inter3[:, 0:MSPLIT], wxr3[:, 0:MSPLIT],
                        wy[:, 0:MSPLIT], op=Alu.mult)
                    nc.vector.tensor_tensor(
                        inter3[:, MSPLIT:P], wxr3[:, MSPLIT:P],
                        wy[:, MSPLIT:P], op=Alu.mult)
                    if with_mask:
                        pen = wk.tile([128, P], F32, tag="wx", name="pen")
                        nc.gpsimd.tensor_scalar(
                            pen[:], invp_t, gscal[:, 40 + c : 41 + c], None,
                            op0=Alu.mult)
                        nc.vector.tensor_tensor(
                            inter3[:], inter3[:], pen[:], op=Alu.subtract)
                        condv = wk.tile([128, P], F32, tag="vx", name="condv")
                    else:
                        condv = wk.tile([128, P], F32, tag="wx", name="condv")
                    nc.vector.scalar_tensor_tensor(
                        condv[:], inter3[:], ag_c[:, c : c + 1], apeps_t,
                        op0=Alu.subtract, op1=Alu.is_gt,
                        accum_out=Scol[:, c : c + 1])
                    for k8 in range(P // 512):
                        nc.tensor.matmul(
                            nt[:, k8 * 512 : (k8 + 1) * 512], ones128[:],
                            condv[:, k8 * 512 : (k8 + 1) * 512],
                            start=(c == 0), stop=(c == NCH - 1))

                # ---------- batch tail ----------
                # num_true = count of gt with >=1 match
                indg = sca.tile([128, NCH], F32, tag="indg", name="indg")
                nc.vector.tensor_scalar(indg[:], Scol[:], 0.0, None, op0=Alu.is_gt)
                nc.vector.tensor_reduce(
                    counts_sb[:, 12 + b : 13 + b], indg[:],
                    axis=mybir.AxisListType.X, op=Alu.add)
                # num_pos = count of preds with >=1 match (colsums exact ints)
                nti = sca.tile([1, P], F32, tag="nti", name="nti")
                nc.scalar.activation(
                    nti[:], nt[:], mybir.ActivationFunctionType.Sign)
                nc.vector.tensor_reduce(
                    counts_sb[0:1, b : b + 1], nti[:],
                    axis=mybir.AxisListType.X, op=Alu.add)

            # ---------- final: sum over partitions, write out ----------
            counts_ps = ps.tile([1, 16], F32, tag="nt", name="cps")
            nc.tensor.matmul(
                counts_ps[:], ones128[:], counts_sb[:], start=True, stop=True)
            counts_out = cst.tile([1, 16], F32)
            nc.vector.tensor_copy(counts_out[:], counts_ps[:])
            nc.sync.dma_start(counts_d[:], counts_out[:])

    nc.compile()
    return nc


def _get_program(with_mask: bool):
    key = (with_mask, REPEAT)
    if key not in _PROGRAM_CACHE:
        _PROGRAM_CACHE[key] = _build(with_mask)
    return _PROGRAM_CACHE[key]


def _run_device(pred, gt, with_mask, trace=False):
    nc = _get_program(with_mask)
    in_maps = [
        {
            "pred": np.ascontiguousarray(pred[i * BPC : (i + 1) * BPC]),
            "gt": np.ascontiguousarray(gt[i * BPC : (i + 1) * BPC]),
        }
        for i in range(N_CORES)
    ]
    res = run_bass_kernel_spmd(nc, in_maps, list(range(N_CORES)), trace=trace)
    counts = np.stack([res.results[i]["counts"][0] for i in range(N_CORES)])
    return counts, res  # counts: [N_CORES, 16]


def kernel(pred_boxes, gt_boxes, _trace=False):
    pred = np.asarray(pred_boxes, dtype=np.float32)
    gt = np.asarray(gt_boxes, dtype=np.float32)
    assert pred.shape == (B_TOTAL, P, 6) and gt.shape == (B_TOTAL, G, 6)

    # the ignore mask only differs from all-ones when a pred AND a gt box are
    # both padding (cx == -1); specialize the program accordingly
    with_mask = bool((pred[..., 0] == -1.0).any() and (gt[..., 0] == -1.0).any())

    counts, res = _run_device(pred, gt, with_mask, trace=_trace)
    kernel.last_results = res

    num_pos = counts[:, 0:4].reshape(-1).astype(np.float32)
    num_true = counts[:, 12:16].reshape(-1).astype(np.float32)
    if with_mask:
        num_pred = counts[:, 4:8].reshape(-1).astype(np.float32)
        num_gt = counts[:, 8:12].reshape(-1).astype(np.float32)
    else:
        # all boxes valid (host-verified): counts are the full box counts
        num_pred = np.full(B_TOTAL, np.float32(P), dtype=np.float32)
        num_gt = np.full(B_TOTAL, np.float32(G), dtype=np.float32)

    eps = np.float32(EPS)
    precision = num_pos / (num_pred + eps)
    recall = num_true / (num_gt + eps)
    fmeasure = np.float32(2.0) * (precision * recall) / (precision + recall + eps)
    return (precision, recall, fmeasure)



# revision 3
# speedup vs baseline: 6.0464x; 6.0464x over previous
"""DetectionIOUMetric Trainium2 kernel.

Computes, for pred_boxes [32, 4096, 6] and gt_boxes [32, 1024, 6] (cx, cy, w, h
in the first 4 channels; a box is padding iff cx == -1):

    masked pairwise IoU, num_pos / num_true / num_pred / num_gt per batch,
    precision / recall / F1 per batch.

Sharding: pure data parallel over the batch dim - each of the 8 NeuronCores
processes 4 batches; no cross-device communication. The device program
computes the integer counts per batch; the final eps-divisions happen on the
host after the gather.

Fast path (no padded boxes, which is the case for uniform random inputs):
fp16 pairwise pipeline. Per batch, gt boxes live on partitions (8 chunks of
128), preds on the free dim (FD=4096). The IoU>0.5 test is algebraically
rearranged (exact in real arithmetic):

    iou > 0.5  <=>  3*inter - area_g > area_p + eps        (union + eps > 0)
    inter      =  relu(wx) * wy  (one-sided relu suffices)
    3*wx is computed directly from x-rows pre-scaled by 3 (monotone min ok)

Per chunk (measured engine costs drive the assignment):
    m1 = min(3px2_t, 3gx2)      DVE ts   (fp16 4x mode, ~1.3us)
    m2 = min(-3px1_t, -3gx1)    DVE ts
    m3 = min(py2_t, gy2)        DVE ts
    m4 = min(-py1_t, -gy1)      DVE ts
    wx3 = m1 + m2               tt, column-split DVE / GpSimd
    wy  = m3 + m4               tt, column-split DVE / GpSimd
    xr  = relu(wx3)             ACT
    inter3 = xr * wy            GpSimd tt
    A = apeps_t + ag            DVE ts
    condv = inter3 > A (fp16)   DVE tt
    Scol[:, c] += rowsum(condv) ACT relu + accum_out
    nt[8,512]  += colsums       PE, one-hot [128,8] stationaries

fp16 rounding of the pipeline was validated against the reference on the
actual input distribution: worst metric rel-err ~2e-3 (tolerance 2e-2).
A fp32 masked-path program (baseline algorithm) is kept as fallback for
inputs that contain padded boxes.
"""
import numpy as np

import concourse.bass as bass
import concourse.bacc as bacc
import concourse.tile as tile
from concourse import mybir
from concourse.bass_utils import run_bass_kernel_spmd

F32 = mybir.dt.float32
F16 = mybir.dt.float16
EPS = 1e-7
IOU_PENALTY = 1e30

B_TOTAL = 32
N_CORES = 8
BPC = B_TOTAL // N_CORES       # batches per core
P = 4096                       # pred boxes per batch (free dim)
G = 1024                       # gt boxes per batch (8 partition chunks)
NCH = G // 128                 # 8 gt chunks per batch
NROW = 5                       # broadcast rows: 3px2, -3px1, py2, -py1, apeps
XSPL = 2816                    # wx3 cols on DVE; rest on GpSimd
YSPL = 3584                    # wy cols on DVE; rest on GpSimd

_PROGRAM_CACHE = {}

Alu = mybir.AluOpType
Act = mybir.ActivationFunctionType


def _build_fast():
    """Unmasked fp16 program: inputs pred [BPC, P, 6] / gt [BPC, G, 6],
    output counts [1, 16]: col b = num_pos[b], col 12+b = num_true[b]."""
    nc = bacc.Bacc(None, target_bir_lowering=False)
    pred_d = nc.dram_tensor("pred", [BPC, P, 6], F32, kind="ExternalInput")
    gt_d = nc.dram_tensor("gt", [BPC, G, 6], F32, kind="ExternalInput")
    counts_d = nc.dram_tensor("counts", [1, 16], F32, kind="ExternalOutput")

    with tile.TileContext(nc) as tc:
        with (
            tc.tile_pool(name="cst", bufs=1) as cst,
            tc.tile_pool(name="rows", bufs=2) as rows,
            tc.tile_pool(name="gtp", bufs=2) as gtp,
            tc.tile_pool(name="sca", bufs=2) as sca,
            tc.tile_pool(name="wk", bufs=2) as wk,
            tc.tile_pool(name="wkd", bufs=1) as wkd,
            tc.tile_pool(name="ps", bufs=2, space=bass.MemorySpace.PSUM) as ps,
            tc.tile_pool(name="dram", bufs=2, space=bass.MemorySpace.DRAM) as dram,
        ):
            ones128 = cst.tile([128, 1], F32)
            nc.vector.memset(ones128[:], 1.0)
            # sel8[:, 8k:8k+8] is the k-th one-hot-column stationary: a
            # matmul with it writes the colsum into row k of nt[8, 512].
            sel8 = cst.tile([128, 64], F16)
            nc.vector.memset(sel8[:], 0.0)
            for k in range(8):
                nc.vector.memset(sel8[:, 8 * k + k : 8 * k + k + 1], 1.0)
            counts_sb = cst.tile([128, 16], F32)
            nc.vector.memset(counts_sb[:], 0.0)
            dummy = wkd.tile([128, P], F16)

            for b in range(BPC):
                # ---------- pred prep ----------
                # [32, 768]: partition q holds pred boxes 128q .. 128q+127
                pred_lin = rows.tile([32, 768], F32)
                nc.sync.dma_start(
                    pred_lin[:],
                    pred_d.ap()[b].rearrange("(q x) c -> q (x c)", q=32),
                )
                r3p = pred_lin[:].rearrange("q (x c) -> q x c", c=6)
                pcx = r3p[:, :, 0]
                pcy = r3p[:, :, 1]
                pw = r3p[:, :, 2]
                ph = r3p[:, :, 3]
                pcx3 = sca.tile([32, 128], F32, tag="pcx3", name="pcx3")
                nc.vector.tensor_scalar(pcx3[:], pcx, 3.0, None, op0=Alu.mult)
                psmall = rows.tile([32, NROW * 128], F16)
                nc.vector.scalar_tensor_tensor(
                    psmall[:, 0:128], pw, 1.5, pcx3[:],
                    op0=Alu.mult, op1=Alu.add)
                nc.vector.scalar_tensor_tensor(
                    psmall[:, 128:256], pw, 1.5, pcx3[:],
                    op0=Alu.mult, op1=Alu.subtract)
                nc.vector.scalar_tensor_tensor(
                    psmall[:, 256:384], ph, 0.5, pcy,
                    op0=Alu.mult, op1=Alu.add)
                nc.vector.scalar_tensor_tensor(
                    psmall[:, 384:512], ph, 0.5, pcy,
                    op0=Alu.mult, op1=Alu.subtract)
                ap32 = sca.tile([32, 128], F32, tag="ap32", name="ap32")
                nc.vector.tensor_tensor(ap32[:], pw, ph, op=Alu.mult)
                nc.vector.tensor_scalar(
                    psmall[:, 512:640], ap32[:], EPS, None, op0=Alu.add)

                # stage to DRAM in pred order: scr[t, 128q+j] = psmall[q, 128t+j]
                scr = dram.tile([NROW, P], F16)
                nc.sync.dma_start(
                    scr[:].rearrange("t (q j) -> q t j", j=128),
                    psmall[:].rearrange("q (t j) -> q t j", j=128),
                )
                # broadcast to all 128 partitions (8 partition-group DMAs,
                # contiguous 20KB inner runs; step-0 only on the rep dim)
                big = gtp.tile([128, NROW * P], F16, tag="big", name="big")
                scr_flat = scr[:].rearrange("t g -> (t g)")
                H = NROW * P // 2
                for g4 in range(4):
                    for h2 in range(2):
                        nc.sync.dma_start(
                            big[g4 * 32 : (g4 + 1) * 32,
                                h2 * H : (h2 + 1) * H],
                            scr_flat[None, None, h2 * H : (h2 + 1) * H]
                            .broadcast_to([1, 32, H]),
                        )
                p3x2_t = big[:, 0 * P : 1 * P]
                m3px1_t = big[:, 1 * P : 2 * P]
                py2_t = big[:, 2 * P : 3 * P]
                mpy1_t = big[:, 3 * P : 4 * P]
                apeps_t = big[:, 4 * P : 5 * P]

                # ---------- gt prep ----------
                # [128, 48]: partition p holds gt boxes 8p .. 8p+7; chunk c
                # pairs partition p with gt box 8p+c (order-invariant counts)
                gt_lin = rows.tile([128, 48], F32)
                nc.sync.dma_start(
                    gt_lin[:], gt_d.ap()[b].rearrange("(q x) c -> q (x c)", q=128)
                )
                r3g = gt_lin[:].rearrange("q (x c) -> q x c", c=6)
                gcx = r3g[:, :, 0]
                gcy = r3g[:, :, 1]
                gw = r3g[:, :, 2]
                gh = r3g[:, :, 3]
                gsc = sca.tile([128, 48], F32, tag="gsc", name="gsc")
                gcx3 = gsc[:, 40:48]
                nc.vector.tensor_scalar(gcx3, gcx, 3.0, None, op0=Alu.mult)
                nc.vector.scalar_tensor_tensor(
                    gsc[:, 0:8], gw, 1.5, gcx3, op0=Alu.mult, op1=Alu.add)
                nc.vector.scalar_tensor_tensor(
                    gsc[:, 8:16], gw, 1.5, gcx3,
                    op0=Alu.mult, op1=Alu.subtract)
                nc.vector.scalar_tensor_tensor(
                    gsc[:, 16:24], gh, 0.5, gcy, op0=Alu.mult, op1=Alu.add)
                nc.vector.scalar_tensor_tensor(
                    gsc[:, 24:32], gh, 0.5, gcy,
                    op0=Alu.mult, op1=Alu.subtract)
                nc.vector.tensor_tensor(gsc[:, 32:40], gw, gh, op=Alu.mult)

                # ---------- chunk loop over 8 gt chunks ----------
                Scol = sca.tile([128, NCH], F32, tag="Scol", name="Scol")
                nt = ps.tile([8, 512], F32, tag="nt", name="nt")
                for c in range(NCH):
                    m1 = wk.tile([128, P], F16, tag="P1", name="m1")
                    nc.vector.tensor_scalar(
                        m1[:], p3x2_t, gsc[:, c : c + 1], None, op0=Alu.min)
                    m2 = wk.tile([128, P], F16, tag="P2", name="m2")
                    nc.vector.tensor_scalar(
                        m2[:], m3px1_t, gsc[:, 8 + c : 9 + c], None, op0=Alu.min)
                    wx3 = wk.tile([128, P], F16, tag="P1", name="wx3")
                    nc.vector.tensor_tensor(
                        wx3[:, 0:XSPL], m1[:, 0:XSPL], m2[:, 0:XSPL], op=Alu.add)
                    nc.gpsimd.tensor_tensor(
                        wx3[:, XSPL:P], m1[:, XSPL:P], m2[:, XSPL:P], op=Alu.add)
                    xr = wk.tile([128, P], F16, tag="P4", name="xr")
                    nc.scalar.activation(xr[:], wx3[:], Act.Relu)
                    m3 = wk.tile([128, P], F16, tag="P3", name="m3")
                    nc.vector.tensor_scalar(
                        m3[:], py2_t, gsc[:, 16 + c : 17 + c], None, op0=Alu.min)
                    m4 = wk.tile([128, P], F16, tag="P2", name="m4")
                    nc.vector.tensor_scalar(
                        m4[:], mpy1_t, gsc[:, 24 + c : 25 + c], None, op0=Alu.min)
                    wy = wk.tile([128, P], F16, tag="P3", name="wy")
                    nc.vector.tensor_tensor(
                        wy[:, 0:YSPL], m3[:, 0:YSPL], m4[:, 0:YSPL], op=Alu.add)
                    nc.gpsimd.tensor_tensor(
                        wy[:, YSPL:P], m3[:, YSPL:P], m4[:, YSPL:P], op=Alu.add)
                    A = wk.tile([128, P], F16, tag="TA", name="A")
                    nc.vector.tensor_scalar(
                        A[:], apeps_t, gsc[:, 32 + c : 33 + c], None, op0=Alu.add)
                    inter3 = wk.tile([128, P], F16, tag="Q", name="inter3")
                    nc.gpsimd.tensor_tensor(
                        inter3[:], xr[:], wy[:], op=Alu.mult)
                    condv = wk.tile([128, P], F16, tag="Q", name="condv")
                    nc.vector.tensor_tensor(
                        condv[:], inter3[:], A[:], op=Alu.is_gt)
                    # per-gt row counts (condv >= 0, relu is identity)
                    nc.scalar.activation(
                        dummy[:], condv[:], Act.Relu,
                        accum_out=Scol[:, c : c + 1])
                    # per-pred col sums into row c%8 of nt via one-hot sel
                    for k8 in range(8):
                        nc.tensor.matmul(
                            nt[:], sel8[:, 8 * k8 : 8 * k8 + 8],
                            condv[:, k8 * 512 : (k8 + 1) * 512],
                            start=(c == 0 and k8 == 0),
                            stop=(c == NCH - 1 and k8 == 7))

                # ---------- batch tail ----------
                vgd = sca.tile([128, NCH], F32, tag="vgd", name="vgd")
                nc.vector.tensor_scalar(
                    vgd[:], Scol[:], 0.0, 0.0, op0=Alu.is_gt, op1=Alu.add,
                    accum_out=counts_sb[:, 12 + b : 13 + b])
                npd = sca.tile([8, 512], F32, tag="npd", name="npd")
                nc.vector.tensor_scalar(
                    npd[:], nt[:], 0.0, 0.0, op0=Alu.is_gt, op1=Alu.add,
                    accum_out=counts_sb[0:8, b : b + 1])

            # ---------- final: sum over partitions, write out ----------
            counts_ps = ps.tile([1, 16], F32, tag="cps", name="cps")
            nc.tensor.matmul(
                counts_ps[:], ones128[:], counts_sb[:], start=True, stop=True)
            counts_out = cst.tile([1, 16], F32)
            nc.vector.tensor_copy(counts_out[:], counts_ps[:])
            nc.sync.dma_start(counts_d[:], counts_out[:])

    nc.compile()
    return nc


def _build_masked():
    """Masked fp32 fallback (baseline algorithm): output counts [1, 16] =
    per-batch [num_pos, num_pred, num_gt, num_true]."""
    MSPLIT = 2560
    nc = bacc.Bacc(None, target_bir_lowering=False)
    pred_d = nc.dram_tensor("pred", [BPC, P, 6], F32, kind="ExternalInput")
    gt_d = nc.dram_tensor("gt", [BPC, G, 6], F32, kind="ExternalInput")
    counts_d = nc.dram_tensor("counts", [1, 16], F32, kind="ExternalOutput")

    with tile.TileContext(nc) as tc:
        with (
            tc.tile_pool(name="cst", bufs=1) as cst,
            tc.tile_pool(name="rows", bufs=2) as rows,
            tc.tile_pool(name="gtp", bufs=1) as gtp,
            tc.tile_pool(name="sca", bufs=2) as sca,
            tc.tile_pool(name="wk", bufs=1) as wk,
            tc.tile_pool(name="ps", bufs=1, space=bass.MemorySpace.PSUM) as ps,
            tc.tile_pool(name="dram", bufs=2, space=bass.MemorySpace.DRAM) as dram,
        ):
            ones128 = cst.tile([128, 1], F32)
            nc.vector.memset(ones128[:], 1.0)
            counts_sb = cst.tile([128, 16], F32)
            nc.vector.memset(counts_sb[:], 0.0)

            for b in range(BPC):
                NROWM = 6
                pred_lin = rows.tile([32, 768], F32)
                nc.sync.dma_start(
                    pred_lin[:],
                    pred_d.ap()[b].rearrange("(q x) c -> q (x c)", q=32),
                )
                r3p = pred_lin[:].rearrange("q (x c) -> q x c", c=6)
                pcx = r3p[:, :, 0]
                pcy = r3p[:, :, 1]
                pw = r3p[:, :, 2]
                ph = r3p[:, :, 3]
                psmall = rows.tile([32, NROWM * 128], F32)
                px2_s = psmall[:, 0:128]
                mpx1_s = psmall[:, 128:256]
                py2_s = psmall[:, 256:384]
                mpy1_s = psmall[:, 384:512]
                apeps_s = psmall[:, 512:640]
                nc.vector.scalar_tensor_tensor(
                    px2_s, pw, 0.5, pcx, op0=Alu.mult, op1=Alu.add)
                nc.vector.scalar_tensor_tensor(
                    mpx1_s, pw, 0.5, pcx, op0=Alu.mult, op1=Alu.subtract)
                nc.vector.scalar_tensor_tensor(
                    py2_s, ph, 0.5, pcy, op0=Alu.mult, op1=Alu.add)
                nc.vector.scalar_tensor_tensor(
                    mpy1_s, ph, 0.5, pcy, op0=Alu.mult, op1=Alu.subtract)
                dx_s = sca.tile([32, 128], F32, tag="dx_s", name="dx_s")
                dy_s = sca.tile([32, 128], F32, tag="dy_s", name="dy_s")
                nc.vector.tensor_tensor(dx_s[:], px2_s, mpx1_s, op=Alu.add)
                nc.vector.tensor_tensor(dy_s[:], py2_s, mpy1_s, op=Alu.add)
                nc.vector.tensor_tensor(apeps_s, dx_s[:], dy_s[:], op=Alu.mult)
                nc.vector.tensor_scalar(
                    apeps_s, apeps_s, EPS, None, op0=Alu.add)
                nc.vector.tensor_scalar(
                    psmall[:, 640:768], pcx, -1.0, None, op0=Alu.is_equal)

                vp = sca.tile([32, 128], F32, tag="vp", name="vp")
                nc.vector.tensor_scalar(
                    vp[:], pcx, -1.0, None, op0=Alu.not_equal)
                nc.vector.tensor_reduce(
                    counts_sb[0:32, 4 + b : 5 + b], vp[:],
                    axis=mybir.AxisListType.X, op=Alu.add)

                scr = dram.tile([NROWM, P], F32)
                nc.sync.dma_start(
                    scr[:].rearrange("t (q j) -> q t j", j=128),
                    psmall[:].rearrange("q (t j) -> q t j", j=128),
                )
                big = gtp.tile([128, NROWM * P], F32, tag="big", name="big")
                scr_flat = scr[:].rearrange("t g -> (t g)")
                H = NROWM * P // 2
                for g4 in range(4):
                    for h2 in range(2):
                        nc.sync.dma_start(
                            big[g4 * 32 : (g4 + 1) * 32,
                                h2 * H : (h2 + 1) * H],
                            scr_flat[None, None, h2 * H : (h2 + 1) * H]
                            .broadcast_to([1, 32, H]),
                        )
                px2_t = big[:, 0 * P : 1 * P]
                mpx1_t = big[:, 1 * P : 2 * P]
                py2_t = big[:, 2 * P : 3 * P]
                mpy1_t = big[:, 3 * P : 4 * P]
                apeps_t = big[:, 4 * P : 5 * P]
                invp_t = big[:, 5 * P : 6 * P]

                gt_lin = rows.tile([128, 48], F32)
                nc.sync.dma_start(
                    gt_lin[:], gt_d.ap()[b].rearrange("(q x) c -> q (x c)", q=128)
                )
                r3g = gt_lin[:].rearrange("q (x c) -> q x c", c=6)
                gcx = r3g[:, :, 0]
                gcy = r3g[:, :, 1]
                gw = r3g[:, :, 2]
                gh = r3g[:, :, 3]
                gscal = sca.tile([128, 48], F32, tag="gscal", name="gscal")
                gx2_c = gscal[:, 0:8]
                mgx1_c = gscal[:, 8:16]
                gy2_c = gscal[:, 16:24]
                mgy1_c = gscal[:, 24:32]
                ag_c = gscal[:, 32:40]
                nc.vector.scalar_tensor_tensor(
                    gx2_c, gw, 0.5, gcx, op0=Alu.mult, op1=Alu.add)
                nc.vector.scalar_tensor_tensor(
                    mgx1_c, gw, 0.5, gcx, op0=Alu.mult, op1=Alu.subtract)
                nc.vector.scalar_tensor_tensor(
                    gy2_c, gh, 0.5, gcy, op0=Alu.mult, op1=Alu.add)
                nc.vector.scalar_tensor_tensor(
                    mgy1_c, gh, 0.5, gcy, op0=Alu.mult, op1=Alu.subtract)
                nc.vector.tensor_tensor(ag_c, gw, gh, op=Alu.mult)
                nc.vector.tensor_scalar(
                    gscal[:, 40:48], gcx, -1.0, IOU_PENALTY,
                    op0=Alu.is_equal, op1=Alu.mult)

                vg = sca.tile([128, 8], F32, tag="vg", name="vg")
                nc.vector.tensor_scalar(
                    vg[:], gcx, -1.0, None, op0=Alu.not_equal)
                nc.vector.tensor_reduce(
                    counts_sb[:, 8 + b : 9 + b], vg[:],
                    axis=mybir.AxisListType.X, op=Alu.add)

                Scol = sca.tile([128, NCH], F32, tag="Scol", name="Scol")
                nt = ps.tile([1, P], F32, tag="nt", name="nt")
                for c in range(NCH):
                    vx = wk.tile([128, P], F32, tag="vx", name="vx")
                    nc.vector.tensor_scalar(
                        vx[:], mpx1_t, mgx1_c[:, c : c + 1], None, op0=Alu.min)
                    wx = wk.tile([128, P], F32, tag="wx", name="wx")
                    nc.vector.scalar_tensor_tensor(
                        wx[:], px2_t, gx2_c[:, c : c + 1], vx[:],
                        op0=Alu.min, op1=Alu.add)
                    vy = wk.tile([128, P], F32, tag="vy", name="vy")
                    nc.vector.tensor_scalar(
                        vy[:], mpy1_t, mgy1_c[:, c : c + 1], None, op0=Alu.min)
                    wy = wk.tile([128, P], F32, tag="wy", name="wy")
                    nc.vector.scalar_tensor_tensor(
                        wy[:], py2_t, gy2_c[:, c : c + 1], vy[:],
                        op0=Alu.min, op1=Alu.add)
                    wxr3 = wk.tile([128, P], F32, tag="vx", name="wxr3")
                    nc.scalar.activation(
                        wxr3[:], wx[:], Act.Relu, scale=3.0)
                    inter3 = wk.tile([128, P], F32, tag="vy", name="inter3")
                    nc.gpsimd.tensor_tensor(
                        inter3[:, 0:MSPLIT], wxr3[:, 0:MSPLIT],
                        wy[:, 0:MSPLIT], op=Alu.mult)
                    nc.vector.tensor_tensor(
                        inter3[:, MSPLIT:P], wxr3[:, MSPLIT:P],
                        wy[:, MSPLIT:P], op=Alu.mult)
                    pen = wk.tile([128, P], F32, tag="wx", name="pen")
                    nc.gpsimd.tensor_scalar(
                        pen[:], invp_t, gscal[:, 40 + c : 41 + c], None,
                        op0=Alu.mult)
                    nc.vector.tensor_tensor(
                        inter3[:], inter3[:], pen[:], op=Alu.subtract)
                    condv = wk.tile([128, P], F32, tag="vx", name="condv")
                    nc.vector.scalar_tensor_tensor(
                        condv[:], inter3[:], ag_c[:, c : c + 1], apeps_t,
                        op0=Alu.subtract, op1=Alu.is_gt,
                        accum_out=Scol[:, c : c + 1])
                    for k8 in range(P // 512):
                        nc.tensor.matmul(
                            nt[:, k8 * 512 : (k8 + 1) * 512], ones128[:],
                            condv[:, k8 * 512 : (k8 + 1) * 512],
                            start=(c == 0), stop=(c == NCH - 1))

                indg = sca.tile([128, NCH], F32, tag="indg", name="indg")
                nc.vector.tensor_scalar(indg[:], Scol[:], 0.0, None, op0=Alu.is_gt)
                nc.vector.tensor_reduce(
                    counts_sb[:, 12 + b : 13 + b], indg[:],
                    axis=mybir.AxisListType.X, op=Alu.add)
                nti = sca.tile([1, P], F32, tag="nti", name="nti")
                nc.scalar.activation(nti[:], nt[:], Act.Sign)
                nc.vector.tensor_reduce(
                    counts_sb[0:1, b : b + 1], nti[:],
                    axis=mybir.AxisListType.X, op=Alu.add)

            counts_ps = ps.tile([1, 16], F32, tag="nt", name="cps")
            nc.tensor.matmul(
                counts_ps[:], ones128[:], counts_sb[:], start=True, stop=True)
            counts_out = cst.tile([1, 16], F32)
            nc.vector.tensor_copy(counts_out[:], counts_ps[:])
            nc.sync.dma_start(counts_d[:], counts_out[:])

    nc.compile()
    return nc


def _get_program(with_mask: bool):
    if with_mask not in _PROGRAM_CACHE:
        _PROGRAM_CACHE[with_mask] = (
            _build_masked() if with_mask else _build_fast()
        )
    return _PROGRAM_CACHE[with_mask]


def _run_device(pred, gt, with_mask, trace=False):
    nc = _get_program(with_mask)
    in_maps = [
        {
            "pred": np.ascontiguousarray(pred[i * BPC : (i + 1) * BPC]),
            "gt": np.ascontiguousarray(gt[i * BPC : (i + 1) * BPC]),
        }
        for i in range(N_CORES)
    ]
    res = run_bass_kernel_spmd(nc, in_maps, list(range(N_CORES)), trace=trace)
    counts = np.stack([res.results[i]["counts"][0] for i in range(N_CORES)])
    return counts, res  # counts: [N_CORES, 16]


def kernel(pred_boxes, gt_boxes, _trace=False):
    pred = np.asarray(pred_boxes, dtype=np.float32)
    gt = np.asarray(gt_boxes, dtype=np.float32)
    assert pred.shape == (B_TOTAL, P, 6) and gt.shape == (B_TOTAL, G, 6)

    # the ignore mask only differs from all-ones when a pred AND a gt box are
    # both padding (cx == -1); specialize the program accordingly
    with_mask = bool((pred[..., 0] == -1.0).any() and (gt[..., 0] == -1.0).any())

    counts, res = _run_device(pred, gt, with_mask, trace=_trace)
    kernel.last_results = res

    num_pos = counts[:, 0:4].reshape(-1).astype(np.float32)
    num_true = counts[:, 12:16].reshape(-1).astype(np.float32)
    if with_mask:
        num_pred = counts[:, 4:8].reshape(-1).astype(np.float32)
        num_gt = counts[:, 8:12].reshape(-1).astype(np.float32)
    else:
        # all boxes valid (host-verified): counts are the full box counts
        num_pred = np.full(B_TOTAL, np.float32(P), dtype=np.float32)
        num_gt = np.full(B_TOTAL, np.float32(G), dtype=np.float32)

    eps = np.float32(EPS)
    precision = num_pos / (num_pred + eps)
    recall = num_true / (num_gt + eps)
    fmeasure = np.float32(2.0) * (precision * recall) / (precision + recall + eps)
    return (precision, recall, fmeasure)


# revision 4
# speedup vs baseline: 9.5871x; 1.5856x over previous
"""DetectionIOUMetric Trainium2 kernel.

Computes, for pred_boxes [32, 4096, 6] and gt_boxes [32, 1024, 6] (cx, cy, w, h
in the first 4 channels; a box is padding iff cx == -1):

    masked pairwise IoU, num_pos / num_true / num_pred / num_gt per batch,
    precision / recall / F1 per batch.

Sharding: pure data parallel over the batch dim - each of the 8 NeuronCores
processes 4 batches; no cross-device communication. The device program
computes the integer counts per batch; the final eps-divisions happen on the
host after the gather.

Fast path (no padded boxes, which is the case for uniform random inputs):
fp16 pairwise pipeline. Per batch, gt boxes live on partitions (8 chunks of
128), preds on the free dim (FD=4096). The IoU>0.5 test is algebraically
rearranged (exact in real arithmetic):

    iou > 0.5  <=>  3*inter - area_g > area_p + eps        (union + eps > 0)
    inter      =  relu(wx) * wy  (one-sided relu suffices)
    3*wx is computed directly from x-rows pre-scaled by 3 (monotone min ok)

Per chunk (measured engine costs drive the assignment):
    m1 = min(3px2_t, 3gx2)      DVE ts   (fp16 4x mode, ~1.3us)
    m2 = min(-3px1_t, -3gx1)    DVE ts
    m3 = min(py2_t, gy2)        DVE ts
    m4 = min(-py1_t, -gy1)      DVE ts
    wx3 = m1 + m2               tt, column-split DVE / GpSimd
    wy  = m3 + m4               tt, column-split DVE / GpSimd
    xr  = relu(wx3)             ACT
    inter3 = xr * wy            GpSimd tt
    A = apeps_t + ag            DVE ts
    condv = inter3 > A (fp16)   DVE tt
    Scol[:, c] += rowsum(condv) ACT relu + accum_out
    nt[8,512]  += colsums       PE, one-hot [128,8] stationaries

fp16 rounding of the pipeline was validated against the reference on the
actual input distribution: worst metric rel-err ~2e-3 (tolerance 2e-2).
A fp32 masked-path program (baseline algorithm) is kept as fallback for
inputs that contain padded boxes.
"""
import numpy as np

import concourse.bass as bass
import concourse.bacc as bacc
import concourse.tile as tile
from concourse import mybir
from concourse.bass_utils import run_bass_kernel_spmd

F32 = mybir.dt.float32
F16 = mybir.dt.float16
EPS = 1e-7
IOU_PENALTY = 1e30

B_TOTAL = 32
N_CORES = 8
BPC = B_TOTAL // N_CORES       # batches per core
P = 4096                       # pred boxes per batch (free dim)
G = 1024                       # gt boxes per batch (8 partition chunks)
NCH = G // 128                 # 8 gt chunks per batch
NROW = 5                       # broadcast rows: 3px2, -3px1, py2, -py1, apeps
XSPL = 2816                    # wx3 cols on DVE; rest on GpSimd
YSPL = 3584                    # wy cols on DVE; rest on GpSimd

_PROGRAM_CACHE = {}

Alu = mybir.AluOpType
Act = mybir.ActivationFunctionType


def _build_fast():
    """Unmasked fp16 program: inputs pred [BPC, P, 6] / gt [BPC, G, 6],
    output counts [1, 16]: col b = num_pos[b], col 12+b = num_true[b]."""
    nc = bacc.Bacc(None, target_bir_lowering=False)
    pred_d = nc.dram_tensor("pred", [BPC, P, 6], F32, kind="ExternalInput")
    gt_d = nc.dram_tensor("gt", [BPC, G, 6], F32, kind="ExternalInput")
    counts_d = nc.dram_tensor("counts", [1, 16], F32, kind="ExternalOutput")

    with tile.TileContext(nc) as tc:
        with (
            tc.tile_pool(name="cst", bufs=1) as cst,
            tc.tile_pool(name="rows", bufs=2) as rows,
            tc.tile_pool(name="gtp", bufs=2) as gtp,
            tc.tile_pool(name="sca", bufs=2) as sca,
            tc.tile_pool(name="wk", bufs=2) as wk,
            tc.tile_pool(name="wkd", bufs=1) as wkd,
            tc.tile_pool(name="ps", bufs=2, space=bass.MemorySpace.PSUM) as ps,
            tc.tile_pool(name="dram", bufs=2, space=bass.MemorySpace.DRAM) as dram,
        ):
            ones128 = cst.tile([128, 1], F32)
            nc.vector.memset(ones128[:], 1.0)
            # sel8[:, 8k:8k+8] is the k-th one-hot-column stationary: a
            # matmul with it writes the colsum into row k of nt[8, 512].
            sel8 = cst.tile([128, 64], F16)
            nc.vector.memset(sel8[:], 0.0)
            for k in range(8):
                nc.vector.memset(sel8[:, 8 * k + k : 8 * k + k + 1], 1.0)
            counts_sb = cst.tile([128, 16], F32)
            nc.vector.memset(counts_sb[:], 0.0)
            dummy = wkd.tile([128, P], F16)

            for b in range(BPC):
                # ---------- pred prep ----------
                # [32, 768]: partition q holds pred boxes 128q .. 128q+127
                pred_lin = rows.tile([32, 768], F32)
                nc.sync.dma_start(
                    pred_lin[:],
                    pred_d.ap()[b].rearrange("(q x) c -> q (x c)", q=32),
                )
                r3p = pred_lin[:].rearrange("q (x c) -> q x c", c=6)
                pcx = r3p[:, :, 0]
                pcy = r3p[:, :, 1]
                pw = r3p[:, :, 2]
                ph = r3p[:, :, 3]
                pcx3 = sca.tile([32, 128], F32, tag="pcx3", name="pcx3")
                nc.vector.tensor_scalar(pcx3[:], pcx, 3.0, None, op0=Alu.mult)
                psmall = rows.tile([32, NROW * 128], F16)
                nc.vector.scalar_tensor_tensor(
                    psmall[:, 0:128], pw, 1.5, pcx3[:],
                    op0=Alu.mult, op1=Alu.add)
                nc.vector.scalar_tensor_tensor(
                    psmall[:, 128:256], pw, 1.5, pcx3[:],
                    op0=Alu.mult, op1=Alu.subtract)
                nc.vector.scalar_tensor_tensor(
                    psmall[:, 256:384], ph, 0.5, pcy,
                    op0=Alu.mult, op1=Alu.add)
                nc.vector.scalar_tensor_tensor(
                    psmall[:, 384:512], ph, 0.5, pcy,
                    op0=Alu.mult, op1=Alu.subtract)
                ap32 = sca.tile([32, 128], F32, tag="ap32", name="ap32")
                nc.vector.tensor_tensor(ap32[:], pw, ph, op=Alu.mult)
                nc.vector.tensor_scalar(
                    psmall[:, 512:640], ap32[:], EPS, None, op0=Alu.add)

                # stage to DRAM in pred order: scr[t, 128q+j] = psmall[q, 128t+j]
                scr = dram.tile([NROW, P], F16)
                nc.sync.dma_start(
                    scr[:].rearrange("t (q j) -> q t j", j=128),
                    psmall[:].rearrange("q (t j) -> q t j", j=128),
                )
                # broadcast to all 128 partitions (8 partition-group DMAs,
                # contiguous 20KB inner runs; step-0 only on the rep dim)
                big = gtp.tile([128, NROW * P], F16, tag="big", name="big")
                scr_flat = scr[:].rearrange("t g -> (t g)")
                H = NROW * P // 2
                for g4 in range(4):
                    for h2 in range(2):
                        nc.sync.dma_start(
                            big[g4 * 32 : (g4 + 1) * 32,
                                h2 * H : (h2 + 1) * H],
                            scr_flat[None, None, h2 * H : (h2 + 1) * H]
                            .broadcast_to([1, 32, H]),
                        )
                p3x2_t = big[:, 0 * P : 1 * P]
                m3px1_t = big[:, 1 * P : 2 * P]
                py2_t = big[:, 2 * P : 3 * P]
                mpy1_t = big[:, 3 * P : 4 * P]
                apeps_t = big[:, 4 * P : 5 * P]

                # ---------- gt prep ----------
                # [128, 48]: partition p holds gt boxes 8p .. 8p+7; chunk c
                # pairs partition p with gt box 8p+c (order-invariant counts)
                gt_lin = rows.tile([128, 48], F32)
                nc.sync.dma_start(
                    gt_lin[:], gt_d.ap()[b].rearrange("(q x) c -> q (x c)", q=128)
                )
                r3g = gt_lin[:].rearrange("q (x c) -> q x c", c=6)
                gcx = r3g[:, :, 0]
                gcy = r3g[:, :, 1]
                gw = r3g[:, :, 2]
                gh = r3g[:, :, 3]
                gsc = sca.tile([128, 48], F32, tag="gsc", name="gsc")
                gcx3 = gsc[:, 40:48]
                nc.vector.tensor_scalar(gcx3, gcx, 3.0, None, op0=Alu.mult)
                nc.vector.scalar_tensor_tensor(
                    gsc[:, 0:8], gw, 1.5, gcx3, op0=Alu.mult, op1=Alu.add)
                nc.vector.scalar_tensor_tensor(
                    gsc[:, 8:16], gw, 1.5, gcx3,
                    op0=Alu.mult, op1=Alu.subtract)
                nc.vector.scalar_tensor_tensor(
                    gsc[:, 16:24], gh, 0.5, gcy, op0=Alu.mult, op1=Alu.add)
                nc.vector.scalar_tensor_tensor(
                    gsc[:, 24:32], gh, 0.5, gcy,
                    op0=Alu.mult, op1=Alu.subtract)
                nc.vector.tensor_tensor(gsc[:, 32:40], gw, gh, op=Alu.mult)

                # ---------- chunk loop over 8 gt chunks ----------
                Scol = sca.tile([128, NCH], F32, tag="Scol", name="Scol")
                nt = ps.tile([8, 512], F32, tag="nt", name="nt")
                for c in range(NCH):
                    m1 = wk.tile([128, P], F16, tag="P1", name="m1")
                    nc.vector.tensor_scalar(
                        m1[:], p3x2_t, gsc[:, c : c + 1], None, op0=Alu.min)
                    m2 = wk.tile([128, P], F16, tag="P2", name="m2")
                    nc.vector.tensor_scalar(
                        m2[:], m3px1_t, gsc[:, 8 + c : 9 + c], None, op0=Alu.min)
                    # GpSimd is kept OUT of the hot loop: its SBUF port is
                    # shared with DVE's 2nd read port (POOL slot), so GpSimd
                    # work degrades DVE's 2x/4x perf modes (measured).
                    wx3 = wk.tile([128, P], F16, tag="P1", name="wx3")
                    nc.vector.tensor_tensor(wx3[:], m1[:], m2[:], op=Alu.add)
                    xr = wk.tile([128, P], F16, tag="P4", name="xr")
                    nc.scalar.activation(xr[:], wx3[:], Act.Relu)
                    m3 = wk.tile([128, P], F16, tag="P3", name="m3")
                    nc.vector.tensor_scalar(
                        m3[:], py2_t, gsc[:, 16 + c : 17 + c], None, op0=Alu.min)
                    m4 = wk.tile([128, P], F16, tag="P2", name="m4")
                    nc.vector.tensor_scalar(
                        m4[:], mpy1_t, gsc[:, 24 + c : 25 + c], None, op0=Alu.min)
                    wy = wk.tile([128, P], F16, tag="P3", name="wy")
                    nc.vector.tensor_tensor(wy[:], m3[:], m4[:], op=Alu.add)
                    # A = apeps + ag on ACT (Relu is identity: A > 0)
                    A = wk.tile([128, P], F16, tag="TA", name="A")
                    nc.scalar.activation(
                        A[:], apeps_t, Act.Relu, bias=gsc[:, 32 + c : 33 + c])
                    inter3 = wk.tile([128, P], F16, tag="Q", name="inter3")
                    nc.vector.tensor_tensor(
                        inter3[:], xr[:], wy[:], op=Alu.mult)
                    condv = wk.tile([128, P], F16, tag="Q", name="condv")
                    nc.vector.tensor_tensor(
                        condv[:], inter3[:], A[:], op=Alu.is_gt)
                    # per-gt row counts (condv >= 0, relu is identity)
                    nc.scalar.activation(
                        dummy[:], condv[:], Act.Relu,
                        accum_out=Scol[:, c : c + 1])
                    # per-pred col sums into row c%8 of nt via one-hot sel
                    for k8 in range(8):
                        nc.tensor.matmul(
                            nt[:], sel8[:, 8 * k8 : 8 * k8 + 8],
                            condv[:, k8 * 512 : (k8 + 1) * 512],
                            start=(c == 0 and k8 == 0),
                            stop=(c == NCH - 1 and k8 == 7))

                # ---------- batch tail ----------
                vgd = sca.tile([128, NCH], F32, tag="vgd", name="vgd")
                nc.vector.tensor_scalar(
                    vgd[:], Scol[:], 0.0, 0.0, op0=Alu.is_gt, op1=Alu.add,
                    accum_out=counts_sb[:, 12 + b : 13 + b])
                npd = sca.tile([8, 512], F32, tag="npd", name="npd")
                nc.vector.tensor_scalar(
                    npd[:], nt[:], 0.0, 0.0, op0=Alu.is_gt, op1=Alu.add,
                    accum_out=counts_sb[0:8, b : b + 1])

            # ---------- final: sum over partitions, write out ----------
            counts_ps = ps.tile([1, 16], F32, tag="cps", name="cps")
            nc.tensor.matmul(
                counts_ps[:], ones128[:], counts_sb[:], start=True, stop=True)
            counts_out = cst.tile([1, 16], F32)
            nc.vector.tensor_copy(counts_out[:], counts_ps[:])
            nc.sync.dma_start(counts_d[:], counts_out[:])

    nc.compile()
    return nc


def _build_masked():
    """Masked fp32 fallback (baseline algorithm): output counts [1, 16] =
    per-batch [num_pos, num_pred, num_gt, num_true]."""
    MSPLIT = 2560
    nc = bacc.Bacc(None, target_bir_lowering=False)
    pred_d = nc.dram_tensor("pred", [BPC, P, 6], F32, kind="ExternalInput")
    gt_d = nc.dram_tensor("gt", [BPC, G, 6], F32, kind="ExternalInput")
    counts_d = nc.dram_tensor("counts", [1, 16], F32, kind="ExternalOutput")

    with tile.TileContext(nc) as tc:
        with (
            tc.tile_pool(name="cst", bufs=1) as cst,
            tc.tile_pool(name="rows", bufs=2) as rows,
            tc.tile_pool(name="gtp", bufs=1) as gtp,
            tc.tile_pool(name="sca", bufs=2) as sca,
            tc.tile_pool(name="wk", bufs=1) as wk,
            tc.tile_pool(name="ps", bufs=1, space=bass.MemorySpace.PSUM) as ps,
            tc.tile_pool(name="dram", bufs=2, space=bass.MemorySpace.DRAM) as dram,
        ):
            ones128 = cst.tile([128, 1], F32)
            nc.vector.memset(ones128[:], 1.0)
            counts_sb = cst.tile([128, 16], F32)
            nc.vector.memset(counts_sb[:], 0.0)

            for b in range(BPC):
                NROWM = 6
                pred_lin = rows.tile([32, 768], F32)
                nc.sync.dma_start(
                    pred_lin[:],
                    pred_d.ap()[b].rearrange("(q x) c -> q (x c)", q=32),
                )
                r3p = pred_lin[:].rearrange("q (x c) -> q x c", c=6)
                pcx = r3p[:, :, 0]
                pcy = r3p[:, :, 1]
                pw = r3p[:, :, 2]
                ph = r3p[:, :, 3]
                psmall = rows.tile([32, NROWM * 128], F32)
                px2_s = psmall[:, 0:128]
                mpx1_s = psmall[:, 128:256]
                py2_s = psmall[:, 256:384]
                mpy1_s = psmall[:, 384:512]
                apeps_s = psmall[:, 512:640]
                nc.vector.scalar_tensor_tensor(
                    px2_s, pw, 0.5, pcx, op0=Alu.mult, op1=Alu.add)
                nc.vector.scalar_tensor_tensor(
                    mpx1_s, pw, 0.5, pcx, op0=Alu.mult, op1=Alu.subtract)
                nc.vector.scalar_tensor_tensor(
                    py2_s, ph, 0.5, pcy, op0=Alu.mult, op1=Alu.add)
                nc.vector.scalar_tensor_tensor(
                    mpy1_s, ph, 0.5, pcy, op0=Alu.mult, op1=Alu.subtract)
                dx_s = sca.tile([32, 128], F32, tag="dx_s", name="dx_s")
                dy_s = sca.tile([32, 128], F32, tag="dy_s", name="dy_s")
                nc.vector.tensor_tensor(dx_s[:], px2_s, mpx1_s, op=Alu.add)
                nc.vector.tensor_tensor(dy_s[:], py2_s, mpy1_s, op=Alu.add)
                nc.vector.tensor_tensor(apeps_s, dx_s[:], dy_s[:], op=Alu.mult)
                nc.vector.tensor_scalar(
                    apeps_s, apeps_s, EPS, None, op0=Alu.add)
                nc.vector.tensor_scalar(
                    psmall[:, 640:768], pcx, -1.0, None, op0=Alu.is_equal)

                vp = sca.tile([32, 128], F32, tag="vp", name="vp")
                nc.vector.tensor_scalar(
                    vp[:], pcx, -1.0, None, op0=Alu.not_equal)
                nc.vector.tensor_reduce(
                    counts_sb[0:32, 4 + b : 5 + b], vp[:],
                    axis=mybir.AxisListType.X, op=Alu.add)

                scr = dram.tile([NROWM, P], F32)
                nc.sync.dma_start(
                    scr[:].rearrange("t (q j) -> q t j", j=128),
                    psmall[:].rearrange("q (t j) -> q t j", j=128),
                )
                big = gtp.tile([128, NROWM * P], F32, tag="big", name="big")
                scr_flat = scr[:].rearrange("t g -> (t g)")
                H = NROWM * P // 2
                for g4 in range(4):
                    for h2 in range(2):
                        nc.sync.dma_start(
                            big[g4 * 32 : (g4 + 1) * 32,
                                h2 * H : (h2 + 1) * H],
                            scr_flat[None, None, h2 * H : (h2 + 1) * H]
                            .broadcast_to([1, 32, H]),
                        )
                px2_t = big[:, 0 * P : 1 * P]
                mpx1_t = big[:, 1 * P : 2 * P]
                py2_t = big[:, 2 * P : 3 * P]
                mpy1_t = big[:, 3 * P : 4 * P]
                apeps_t = big[:, 4 * P : 5 * P]
                invp_t = big[:, 5 * P : 6 * P]

                gt_lin = rows.tile([128, 48], F32)
                nc.sync.dma_start(
                    gt_lin[:], gt_d.ap()[b].rearrange("(q x) c -> q (x c)", q=128)
                )
                r3g = gt_lin[:].rearrange("q (x c) -> q x c", c=6)
                gcx = r3g[:, :, 0]
                gcy = r3g[:, :, 1]
                gw = r3g[:, :, 2]
                gh = r3g[:, :, 3]
                gscal = sca.tile([128, 48], F32, tag="gscal", name="gscal")
                gx2_c = gscal[:, 0:8]
                mgx1_c = gscal[:, 8:16]
                gy2_c = gscal[:, 16:24]
                mgy1_c = gscal[:, 24:32]
                ag_c = gscal[:, 32:40]
                nc.vector.scalar_tensor_tensor(
                    gx2_c, gw, 0.5, gcx, op0=Alu.mult, op1=Alu.add)
                nc.vector.scalar_tensor_tensor(
                    mgx1_c, gw, 0.5, gcx, op0=Alu.mult, op1=Alu.subtract)
                nc.vector.scalar_tensor_tensor(
                    gy2_c, gh, 0.5, gcy, op0=Alu.mult, op1=Alu.add)
                nc.vector.scalar_tensor_tensor(
                    mgy1_c, gh, 0.5, gcy, op0=Alu.mult, op1=Alu.subtract)
                nc.vector.tensor_tensor(ag_c, gw, gh, op=Alu.mult)
                nc.vector.tensor_scalar(
                    gscal[:, 40:48], gcx, -1.0, IOU_PENALTY,
                    op0=Alu.is_equal, op1=Alu.mult)

                vg = sca.tile([128, 8], F32, tag="vg", name="vg")
                nc.vector.tensor_scalar(
                    vg[:], gcx, -1.0, None, op0=Alu.not_equal)
                nc.vector.tensor_reduce(
                    counts_sb[:, 8 + b : 9 + b], vg[:],
                    axis=mybir.AxisListType.X, op=Alu.add)

                Scol = sca.tile([128, NCH], F32, tag="Scol", name="Scol")
                nt = ps.tile([1, P], F32, tag="nt", name="nt")
                for c in range(NCH):
                    vx = wk.tile([128, P], F32, tag="vx", name="vx")
                    nc.vector.tensor_scalar(
                        vx[:], mpx1_t, mgx1_c[:, c : c + 1], None, op0=Alu.min)
                    wx = wk.tile([128, P], F32, tag="wx", name="wx")
                    nc.vector.scalar_tensor_tensor(
                        wx[:], px2_t, gx2_c[:, c : c + 1], vx[:],
                        op0=Alu.min, op1=Alu.add)
                    vy = wk.tile([128, P], F32, tag="vy", name="vy")
                    nc.vector.tensor_scalar(
                        vy[:], mpy1_t, mgy1_c[:, c : c + 1], None, op0=Alu.min)
                    wy = wk.tile([128, P], F32, tag="wy", name="wy")
                    nc.vector.scalar_tensor_tensor(
                        wy[:], py2_t, gy2_c[:, c : c + 1], vy[:],
                        op0=Alu.min, op1=Alu.add)
                    wxr3 = wk.tile([128, P], F32, tag="vx", name="wxr3")
                    nc.scalar.activation(
                        wxr3[:], wx[:], Act.Relu, scale=3.0)
                    inter3 = wk.tile([128, P], F32, tag="vy", name="inter3")
                    nc.gpsimd.tensor_tensor(
                        inter3[:, 0:MSPLIT], wxr3[:, 0:MSPLIT],
                        wy[:, 0:MSPLIT], op=Alu.mult)
                    nc.vector.tensor_tensor(
                        inter3[:, MSPLIT:P], wxr3[:, MSPLIT:P],
                        wy[:, MSPLIT:P], op=Alu.mult)
                    pen = wk.tile([128, P], F32, tag="wx", name="pen")
                    nc.gpsimd.tensor_scalar(
                        pen[:], invp_t, gscal[:, 40 + c : 41 + c], None,
                        op0=Alu.mult)
                    nc.vector.tensor_tensor(
                        inter3[:], inter3[:], pen[:], op=Alu.subtract)
                    condv = wk.tile([128, P], F32, tag="vx", name="condv")
                    nc.vector.scalar_tensor_tensor(
                        condv[:], inter3[:], ag_c[:, c : c + 1], apeps_t,
                        op0=Alu.subtract, op1=Alu.is_gt,
                        accum_out=Scol[:, c : c + 1])
                    for k8 in range(P // 512):
                        nc.tensor.matmul(
                            nt[:, k8 * 512 : (k8 + 1) * 512], ones128[:],
                            condv[:, k8 * 512 : (k8 + 1) * 512],
                            start=(c == 0), stop=(c == NCH - 1))

                indg = sca.tile([128, NCH], F32, tag="indg", name="indg")
                nc.vector.tensor_scalar(indg[:], Scol[:], 0.0, None, op0=Alu.is_gt)
                nc.vector.tensor_reduce(
                    counts_sb[:, 12 + b : 13 + b], indg[:],
                    axis=mybir.AxisListType.X, op=Alu.add)
                nti = sca.tile([1, P], F32, tag="nti", name="nti")
                nc.scalar.activation(nti[:], nt[:], Act.Sign)
                nc.vector.tensor_reduce(
                    counts_sb[0:1, b : b + 1], nti[:],
                    axis=mybir.AxisListType.X, op=Alu.add)

            counts_ps = ps.tile([1, 16], F32, tag="nt", name="cps")
            nc.tensor.matmul(
                counts_ps[:], ones128[:], counts_sb[:], start=True, stop=True)
            counts_out = cst.tile([1, 16], F32)
            nc.vector.tensor_copy(counts_out[:], counts_ps[:])
            nc.sync.dma_start(counts_d[:], counts_out[:])

    nc.compile()
    return nc


def _get_program(with_mask: bool):
    if with_mask not in _PROGRAM_CACHE:
        _PROGRAM_CACHE[with_mask] = (
            _build_masked() if with_mask else _build_fast()
        )
    return _PROGRAM_CACHE[with_mask]


def _run_device(pred, gt, with_mask, trace=False):
    nc = _get_program(with_mask)
    in_maps = [
        {
            "pred": np.ascontiguousarray(pred[i * BPC : (i + 1) * BPC]),
            "gt": np.ascontiguousarray(gt[i * BPC : (i + 1) * BPC]),
        }
        for i in range(N_CORES)
    ]
    res = run_bass_kernel_spmd(nc, in_maps, list(range(N_CORES)), trace=trace)
    counts = np.stack([res.results[i]["counts"][0] for i in range(N_CORES)])
    return counts, res  # counts: [N_CORES, 16]


def kernel(pred_boxes, gt_boxes, _trace=False):
    pred = np.asarray(pred_boxes, dtype=np.float32)
    gt = np.asarray(gt_boxes, dtype=np.float32)
    assert pred.shape == (B_TOTAL, P, 6) and gt.shape == (B_TOTAL, G, 6)

    # the ignore mask only differs from all-ones when a pred AND a gt box are
    # both padding (cx == -1); specialize the program accordingly
    with_mask = bool((pred[..., 0] == -1.0).any() and (gt[..., 0] == -1.0).any())

    counts, res = _run_device(pred, gt, with_mask, trace=_trace)
    kernel.last_results = res

    num_pos = counts[:, 0:4].reshape(-1).astype(np.float32)
    num_true = counts[:, 12:16].reshape(-1).astype(np.float32)
    if with_mask:
        num_pred = counts[:, 4:8].reshape(-1).astype(np.float32)
        num_gt = counts[:, 8:12].reshape(-1).astype(np.float32)
    else:
        # all boxes valid (host-verified): counts are the full box counts
        num_pred = np.full(B_TOTAL, np.float32(P), dtype=np.float32)
        num_gt = np.full(B_TOTAL, np.float32(G), dtype=np.float32)

    eps = np.float32(EPS)
    precision = num_pos / (num_pred + eps)
    recall = num_true / (num_gt + eps)
    fmeasure = np.float32(2.0) * (precision * recall) / (precision + recall + eps)
    return (precision, recall, fmeasure)


# revision 10
# speedup vs baseline: 11.8354x; 1.2345x over previous
"""DetectionIOUMetric Trainium2 kernel.

Computes, for pred_boxes [32, 4096, 6] and gt_boxes [32, 1024, 6] (cx, cy, w, h
in the first 4 channels; a box is padding iff cx == -1):

    masked pairwise IoU, num_pos / num_true / num_pred / num_gt per batch,
    precision / recall / F1 per batch.

Sharding: pure data parallel over the batch dim - each of the 8 NeuronCores
processes 4 batches; no cross-device communication. The device program
computes the integer counts per batch; the final eps-divisions happen on the
host after the gather.

Fast path (no padded boxes, which is the case for uniform random inputs):
fp16 pairwise pipeline. Per batch, gt boxes live on partitions (8 chunks of
128), preds on the free dim (FD=4096). The IoU>0.5 test is algebraically
rearranged (exact in real arithmetic):

    iou > 0.5  <=>  3*inter - area_g > area_p + eps        (union + eps > 0)
    inter      =  relu(wx) * wy  (one-sided relu suffices)
    3*wx is computed directly from x-rows pre-scaled by 3 (monotone min ok)

Per chunk (measured engine costs drive the assignment):
    m1 = min(3px2_t, 3gx2)      DVE ts   (fp16 4x mode, ~1.3us)
    m2 = min(-3px1_t, -3gx1)    DVE ts
    m3 = min(py2_t, gy2)        DVE ts
    m4 = min(-py1_t, -gy1)      DVE ts
    wx3 = m1 + m2               tt, column-split DVE / GpSimd
    wy  = m3 + m4               tt, column-split DVE / GpSimd
    xr  = relu(wx3)             ACT
    inter3 = xr * wy            GpSimd tt
    A = apeps_t + ag            DVE ts
    condv = inter3 > A (fp16)   DVE tt
    Scol[:, c] += rowsum(condv) ACT relu + accum_out
    nt[8,512]  += colsums       PE, one-hot [128,8] stationaries

fp16 rounding of the pipeline was validated against the reference on the
actual input distribution: worst metric rel-err ~2e-3 (tolerance 2e-2).
A fp32 masked-path program (baseline algorithm) is kept as fallback for
inputs that contain padded boxes.
"""
import numpy as np

import concourse.bass as bass
import concourse.bacc as bacc
import concourse.tile as tile
from concourse import mybir
from concourse.bass_utils import run_bass_kernel_spmd

F32 = mybir.dt.float32
F16 = mybir.dt.float16
EPS = 1e-7
IOU_PENALTY = 1e30

B_TOTAL = 32
N_CORES = 8
BPC = B_TOTAL // N_CORES       # batches per core
P = 4096                       # pred boxes per batch (free dim)
G = 1024                       # gt boxes per batch (8 partition chunks)
NCH = G // 128                 # 8 gt chunks per batch
NROW = 5                       # broadcast rows: 3px2, -3px1, py2, -py1, apeps
PG = 1024                      # pred column-group width for area pruning
NPG = P // PG                  # pred column groups

_PROGRAM_CACHE = {}

Alu = mybir.AluOpType
Act = mybir.ActivationFunctionType


def _build_fast(masks):
    """Unmasked fp16 program: inputs pred [BPC, P, 6] / gt [BPC, G, 6],
    output counts [1, 16]: col b = num_pos[b], col 12+b = num_true[b].

    masks[b][c][pg]: process gt-chunk c x pred-column-group pg of batch b.
    The host sorts preds and gts by area, so tiles whose area ranges are
    more than 2x apart can never contain an IoU>0.5 pair (inter <= min area,
    union >= max area) and are statically skipped.
    """
    nc = bacc.Bacc(None, target_bir_lowering=False)
    pred_d = nc.dram_tensor("pred", [BPC, P, 6], F32, kind="ExternalInput")
    gt_d = nc.dram_tensor("gt", [BPC, G, 6], F32, kind="ExternalInput")
    counts_d = nc.dram_tensor("counts", [1, 16], F32, kind="ExternalOutput")

    with tile.TileContext(nc) as tc:
        with (
            tc.tile_pool(name="cst", bufs=1) as cst,
            tc.tile_pool(name="rows", bufs=2) as rows,
            tc.tile_pool(name="gtp", bufs=2) as gtp,
            tc.tile_pool(name="sca", bufs=2) as sca,
            tc.tile_pool(name="wk", bufs=2) as wk,
            tc.tile_pool(name="wkd", bufs=1) as wkd,
            tc.tile_pool(name="ps", bufs=2, space=bass.MemorySpace.PSUM) as ps,
            tc.tile_pool(name="dram", bufs=2, space=bass.MemorySpace.DRAM) as dram,
        ):
            ones128 = cst.tile([128, 1], F32)
            nc.vector.memset(ones128[:], 1.0)
            # sel8[:, 8k:8k+8] is the k-th one-hot-column stationary: a
            # matmul with it writes the colsum into row k of nt[8, 512].
            sel8 = cst.tile([128, 64], F16)
            nc.vector.memset(sel8[:], 0.0)
            for k in range(8):
                nc.vector.memset(sel8[:, 8 * k + k : 8 * k + k + 1], 1.0)
            counts_sb = cst.tile([128, 16], F32)
            nc.vector.memset(counts_sb[:], 0.0)
            dummy = wkd.tile([128, PG], F16)

            for b in range(BPC):
                # ---------- pred prep ----------
                # [32, 768]: partition q holds pred boxes 128q .. 128q+127
                pred_lin = rows.tile([32, 768], F32)
                nc.sync.dma_start(
                    pred_lin[:],
                    pred_d.ap()[b].rearrange("(q x) c -> q (x c)", q=32),
                )
                r3p = pred_lin[:].rearrange("q (x c) -> q x c", c=6)
                pcx = r3p[:, :, 0]
                pcy = r3p[:, :, 1]
                pw = r3p[:, :, 2]
                ph = r3p[:, :, 3]
                pcx3 = sca.tile([32, 128], F32, tag="pcx3", name="pcx3")
                nc.vector.tensor_scalar(pcx3[:], pcx, 3.0, None, op0=Alu.mult)
                psmall = rows.tile([32, NROW * 128], F16)
                nc.vector.scalar_tensor_tensor(
                    psmall[:, 0:128], pw, 1.5, pcx3[:],
                    op0=Alu.mult, op1=Alu.add)
                nc.vector.scalar_tensor_tensor(
                    psmall[:, 128:256], pw, 1.5, pcx3[:],
                    op0=Alu.mult, op1=Alu.subtract)
                nc.vector.scalar_tensor_tensor(
                    psmall[:, 256:384], ph, 0.5, pcy,
                    op0=Alu.mult, op1=Alu.add)
                nc.vector.scalar_tensor_tensor(
                    psmall[:, 384:512], ph, 0.5, pcy,
                    op0=Alu.mult, op1=Alu.subtract)
                ap32 = sca.tile([32, 128], F32, tag="ap32", name="ap32")
                nc.vector.tensor_tensor(ap32[:], pw, ph, op=Alu.mult)
                nc.vector.tensor_scalar(
                    psmall[:, 512:640], ap32[:], EPS, None, op0=Alu.add)

                # stage to DRAM in pred order: scr[t, 128q+j] = psmall[q, 128t+j]
                scr = dram.tile([NROW, P], F16)
                nc.sync.dma_start(
                    scr[:].rearrange("t (q j) -> q t j", j=128),
                    psmall[:].rearrange("q (t j) -> q t j", j=128),
                )
                # broadcast to all 128 partitions (8 partition-group DMAs,
                # contiguous 20KB inner runs; step-0 only on the rep dim)
                big = gtp.tile([128, NROW * P], F16, tag="big", name="big")
                scr_flat = scr[:].rearrange("t g -> (t g)")
                H = NROW * P // 2
                for g4 in range(4):
                    for h2 in range(2):
                        nc.sync.dma_start(
                            big[g4 * 32 : (g4 + 1) * 32,
                                h2 * H : (h2 + 1) * H],
                            scr_flat[None, None, h2 * H : (h2 + 1) * H]
                            .broadcast_to([1, 32, H]),
                        )
                p3x2_t = big[:, 0 * P : 1 * P]
                m3px1_t = big[:, 1 * P : 2 * P]
                py2_t = big[:, 2 * P : 3 * P]
                mpy1_t = big[:, 3 * P : 4 * P]
                apeps_t = big[:, 4 * P : 5 * P]

                # ---------- gt prep ----------
                # [128, 48]: partition p holds gt boxes 8p .. 8p+7; chunk c
                # pairs partition p with gt box 8p+c (order-invariant counts)
                gt_lin = rows.tile([128, 48], F32)
                nc.sync.dma_start(
                    gt_lin[:], gt_d.ap()[b].rearrange("(q x) c -> q (x c)", q=128)
                )
                r3g = gt_lin[:].rearrange("q (x c) -> q x c", c=6)
                gcx = r3g[:, :, 0]
                gcy = r3g[:, :, 1]
                gw = r3g[:, :, 2]
                gh = r3g[:, :, 3]
                gsc = sca.tile([128, 48], F32, tag="gsc", name="gsc")
                gcx3 = gsc[:, 40:48]
                nc.vector.tensor_scalar(gcx3, gcx, 3.0, None, op0=Alu.mult)
                nc.vector.scalar_tensor_tensor(
                    gsc[:, 0:8], gw, 1.5, gcx3, op0=Alu.mult, op1=Alu.add)
                nc.vector.scalar_tensor_tensor(
                    gsc[:, 8:16], gw, 1.5, gcx3,
                    op0=Alu.mult, op1=Alu.subtract)
                nc.vector.scalar_tensor_tensor(
                    gsc[:, 16:24], gh, 0.5, gcy, op0=Alu.mult, op1=Alu.add)
                nc.vector.scalar_tensor_tensor(
                    gsc[:, 24:32], gh, 0.5, gcy,
                    op0=Alu.mult, op1=Alu.subtract)
                nc.vector.tensor_tensor(gsc[:, 32:40], gw, gh, op=Alu.mult)

                # ---------- tile loop: 8 gt chunks x 4 pred col groups ----------
                Scol = sca.tile([128, NCH * NPG], F32, tag="Scol", name="Scol")
                nc.vector.memset(Scol[:], 0.0)
                nt = ps.tile([8, 512], F32, tag="nt", name="nt")
                tiles = [(c, pg) for c in range(NCH) for pg in range(NPG)
                         if masks[b][c][pg]]
                nmm = sum(1 for _ in tiles) * (PG // 512)
                imm = 0
                for c, pg in tiles:
                    sl = slice(PG * pg, PG * (pg + 1))
                    m1 = wk.tile([128, PG], F16, tag="P1", name="m1")
                    nc.vector.tensor_scalar(
                        m1[:], p3x2_t[:, sl], gsc[:, c : c + 1], None, op0=Alu.min)
                    m2 = wk.tile([128, PG], F16, tag="P2", name="m2")
                    nc.vector.tensor_scalar(
                        m2[:], m3px1_t[:, sl], gsc[:, 8 + c : 9 + c], None,
                        op0=Alu.min)
                    # GpSimd is kept OUT of the hot loop: its SBUF port is
                    # shared with DVE's 2nd read port (POOL slot), so GpSimd
                    # work degrades DVE's 2x/4x perf modes (measured).
                    wx3 = wk.tile([128, PG], F16, tag="P1", name="wx3")
                    nc.vector.tensor_tensor(wx3[:], m1[:], m2[:], op=Alu.add)
                    xr = wk.tile([128, PG], F16, tag="P4", name="xr")
                    nc.scalar.activation(xr[:], wx3[:], Act.Relu)
                    m3 = wk.tile([128, PG], F16, tag="P3", name="m3")
                    nc.vector.tensor_scalar(
                        m3[:], py2_t[:, sl], gsc[:, 16 + c : 17 + c], None,
                        op0=Alu.min)
                    m4 = wk.tile([128, PG], F16, tag="P2", name="m4")
                    nc.vector.tensor_scalar(
                        m4[:], mpy1_t[:, sl], gsc[:, 24 + c : 25 + c], None,
                        op0=Alu.min)
                    wy = wk.tile([128, PG], F16, tag="P3", name="wy")
                    nc.vector.tensor_tensor(wy[:], m3[:], m4[:], op=Alu.add)
                    # A = apeps + ag on ACT (Relu is identity: A > 0)
                    A = wk.tile([128, PG], F16, tag="TA", name="A")
                    nc.scalar.activation(
                        A[:], apeps_t[:, sl], Act.Relu,
                        bias=gsc[:, 32 + c : 33 + c])
                    inter3 = wk.tile([128, PG], F16, tag="Q", name="inter3")
                    nc.vector.tensor_tensor(
                        inter3[:], xr[:], wy[:], op=Alu.mult)
                    condv = wk.tile([128, PG], F16, tag="Q", name="condv")
                    nc.vector.tensor_tensor(
                        condv[:], inter3[:], A[:], op=Alu.is_gt)
                    # per-gt row counts (condv >= 0, relu is identity)
                    nc.scalar.activation(
                        dummy[:], condv[:], Act.Relu,
                        accum_out=Scol[:, NPG * c + pg : NPG * c + pg + 1])
                    # per-pred col sums into row (2*pg+half) of nt
                    for half in range(PG // 512):
                        k8 = (PG // 512) * pg + half
                        nc.tensor.matmul(
                            nt[:], sel8[:, 8 * k8 : 8 * k8 + 8],
                            condv[:, half * 512 : (half + 1) * 512],
                            start=(imm == 0), stop=(imm == nmm - 1))
                        imm += 1

                # ---------- batch tail ----------
                # per-gt any-match: sum the NPG group columns of each chunk
                # (counts >= 0), then count chunks with sum > 0
                s3 = Scol[:].rearrange("p (c g) -> p c g", g=NPG)
                sch = sca.tile([128, 3 * NCH], F32, tag="sch", name="sch")
                nc.vector.tensor_tensor(
                    sch[:, 0:NCH], s3[:, :, 0], s3[:, :, 1], op=Alu.add)
                nc.vector.tensor_tensor(
                    sch[:, NCH : 2 * NCH], s3[:, :, 2], s3[:, :, 3], op=Alu.add)
                nc.vector.tensor_tensor(
                    sch[:, 2 * NCH : 3 * NCH], sch[:, 0:NCH],
                    sch[:, NCH : 2 * NCH], op=Alu.add)
                vgd = sca.tile([128, NCH], F32, tag="vgd", name="vgd")
                nc.vector.tensor_scalar(
                    vgd[:], sch[:, 2 * NCH : 3 * NCH], 0.0, 0.0,
                    op0=Alu.is_gt, op1=Alu.add,
                    accum_out=counts_sb[:, 12 + b : 13 + b])
                npd = sca.tile([8, 512], F32, tag="npd", name="npd")
                nc.vector.tensor_scalar(
                    npd[:], nt[:], 0.0, 0.0, op0=Alu.is_gt, op1=Alu.add,
                    accum_out=counts_sb[0:8, b : b + 1])

            # ---------- final: sum over partitions, write out ----------
            counts_ps = ps.tile([1, 16], F32, tag="cps", name="cps")
            nc.tensor.matmul(
                counts_ps[:], ones128[:], counts_sb[:], start=True, stop=True)
            counts_out = cst.tile([1, 16], F32)
            nc.vector.tensor_copy(counts_out[:], counts_ps[:])
            nc.sync.dma_start(counts_d[:], counts_out[:])

    nc.compile()
    return nc


def _build_masked():
    """Masked fp32 fallback (baseline algorithm): output counts [1, 16] =
    per-batch [num_pos, num_pred, num_gt, num_true]."""
    MSPLIT = 2560
    nc = bacc.Bacc(None, target_bir_lowering=False)
    pred_d = nc.dram_tensor("pred", [BPC, P, 6], F32, kind="ExternalInput")
    gt_d = nc.dram_tensor("gt", [BPC, G, 6], F32, kind="ExternalInput")
    counts_d = nc.dram_tensor("counts", [1, 16], F32, kind="ExternalOutput")

    with tile.TileContext(nc) as tc:
        with (
            tc.tile_pool(name="cst", bufs=1) as cst,
            tc.tile_pool(name="rows", bufs=2) as rows,
            tc.tile_pool(name="gtp", bufs=1) as gtp,
            tc.tile_pool(name="sca", bufs=2) as sca,
            tc.tile_pool(name="wk", bufs=1) as wk,
            tc.tile_pool(name="ps", bufs=1, space=bass.MemorySpace.PSUM) as ps,
            tc.tile_pool(name="dram", bufs=2, space=bass.MemorySpace.DRAM) as dram,
        ):
            ones128 = cst.tile([128, 1], F32)
            nc.vector.memset(ones128[:], 1.0)
            counts_sb = cst.tile([128, 16], F32)
            nc.vector.memset(counts_sb[:], 0.0)

            for b in range(BPC):
                NROWM = 6
                pred_lin = rows.tile([32, 768], F32)
                nc.sync.dma_start(
                    pred_lin[:],
                    pred_d.ap()[b].rearrange("(q x) c -> q (x c)", q=32),
                )
                r3p = pred_lin[:].rearrange("q (x c) -> q x c", c=6)
                pcx = r3p[:, :, 0]
                pcy = r3p[:, :, 1]
                pw = r3p[:, :, 2]
                ph = r3p[:, :, 3]
                psmall = rows.tile([32, NROWM * 128], F32)
                px2_s = psmall[:, 0:128]
                mpx1_s = psmall[:, 128:256]
                py2_s = psmall[:, 256:384]
                mpy1_s = psmall[:, 384:512]
                apeps_s = psmall[:, 512:640]
                nc.vector.scalar_tensor_tensor(
                    px2_s, pw, 0.5, pcx, op0=Alu.mult, op1=Alu.add)
                nc.vector.scalar_tensor_tensor(
                    mpx1_s, pw, 0.5, pcx, op0=Alu.mult, op1=Alu.subtract)
                nc.vector.scalar_tensor_tensor(
                    py2_s, ph, 0.5, pcy, op0=Alu.mult, op1=Alu.add)
                nc.vector.scalar_tensor_tensor(
                    mpy1_s, ph, 0.5, pcy, op0=Alu.mult, op1=Alu.subtract)
                dx_s = sca.tile([32, 128], F32, tag="dx_s", name="dx_s")
                dy_s = sca.tile([32, 128], F32, tag="dy_s", name="dy_s")
                nc.vector.tensor_tensor(dx_s[:], px2_s, mpx1_s, op=Alu.add)
                nc.vector.tensor_tensor(dy_s[:], py2_s, mpy1_s, op=Alu.add)
                nc.vector.tensor_tensor(apeps_s, dx_s[:], dy_s[:], op=Alu.mult)
                nc.vector.tensor_scalar(
                    apeps_s, apeps_s, EPS, None, op0=Alu.add)
                nc.vector.tensor_scalar(
                    psmall[:, 640:768], pcx, -1.0, None, op0=Alu.is_equal)

                vp = sca.tile([32, 128], F32, tag="vp", name="vp")
                nc.vector.tensor_scalar(
                    vp[:], pcx, -1.0, None, op0=Alu.not_equal)
                nc.vector.tensor_reduce(
                    counts_sb[0:32, 4 + b : 5 + b], vp[:],
                    axis=mybir.AxisListType.X, op=Alu.add)

                scr = dram.tile([NROWM, P], F32)
                nc.sync.dma_start(
                    scr[:].rearrange("t (q j) -> q t j", j=128),
                    psmall[:].rearrange("q (t j) -> q t j", j=128),
                )
                big = gtp.tile([128, NROWM * P], F32, tag="big", name="big")
                scr_flat = scr[:].rearrange("t g -> (t g)")
                H = NROWM * P // 2
                for g4 in range(4):
                    for h2 in range(2):
                        nc.sync.dma_start(
                            big[g4 * 32 : (g4 + 1) * 32,
                                h2 * H : (h2 + 1) * H],
                            scr_flat[None, None, h2 * H : (h2 + 1) * H]
                            .broadcast_to([1, 32, H]),
                        )
                px2_t = big[:, 0 * P : 1 * P]
                mpx1_t = big[:, 1 * P : 2 * P]
                py2_t = big[:, 2 * P : 3 * P]
                mpy1_t = big[:, 3 * P : 4 * P]
                apeps_t = big[:, 4 * P : 5 * P]
                invp_t = big[:, 5 * P : 6 * P]

                gt_lin = rows.tile([128, 48], F32)
                nc.sync.dma_start(
                    gt_lin[:], gt_d.ap()[b].rearrange("(q x) c -> q (x c)", q=128)
                )
                r3g = gt_lin[:].rearrange("q (x c) -> q x c", c=6)
                gcx = r3g[:, :, 0]
                gcy = r3g[:, :, 1]
                gw = r3g[:, :, 2]
                gh = r3g[:, :, 3]
                gscal = sca.tile([128, 48], F32, tag="gscal", name="gscal")
                gx2_c = gscal[:, 0:8]
                mgx1_c = gscal[:, 8:16]
                gy2_c = gscal[:, 16:24]
                mgy1_c = gscal[:, 24:32]
                ag_c = gscal[:, 32:40]
                nc.vector.scalar_tensor_tensor(
                    gx2_c, gw, 0.5, gcx, op0=Alu.mult, op1=Alu.add)
                nc.vector.scalar_tensor_tensor(
                    mgx1_c, gw, 0.5, gcx, op0=Alu.mult, op1=Alu.subtract)
                nc.vector.scalar_tensor_tensor(
                    gy2_c, gh, 0.5, gcy, op0=Alu.mult, op1=Alu.add)
                nc.vector.scalar_tensor_tensor(
                    mgy1_c, gh, 0.5, gcy, op0=Alu.mult, op1=Alu.subtract)
                nc.vector.tensor_tensor(ag_c, gw, gh, op=Alu.mult)
                nc.vector.tensor_scalar(
                    gscal[:, 40:48], gcx, -1.0, IOU_PENALTY,
                    op0=Alu.is_equal, op1=Alu.mult)

                vg = sca.tile([128, 8], F32, tag="vg", name="vg")
                nc.vector.tensor_scalar(
                    vg[:], gcx, -1.0, None, op0=Alu.not_equal)
                nc.vector.tensor_reduce(
                    counts_sb[:, 8 + b : 9 + b], vg[:],
                    axis=mybir.AxisListType.X, op=Alu.add)

                Scol = sca.tile([128, NCH], F32, tag="Scol", name="Scol")
                nt = ps.tile([1, P], F32, tag="nt", name="nt")
                for c in range(NCH):
                    vx = wk.tile([128, P], F32, tag="vx", name="vx")
                    nc.vector.tensor_scalar(
                        vx[:], mpx1_t, mgx1_c[:, c : c + 1], None, op0=Alu.min)
                    wx = wk.tile([128, P], F32, tag="wx", name="wx")
                    nc.vector.scalar_tensor_tensor(
                        wx[:], px2_t, gx2_c[:, c : c + 1], vx[:],
                        op0=Alu.min, op1=Alu.add)
                    vy = wk.tile([128, P], F32, tag="vy", name="vy")
                    nc.vector.tensor_scalar(
                        vy[:], mpy1_t, mgy1_c[:, c : c + 1], None, op0=Alu.min)
                    wy = wk.tile([128, P], F32, tag="wy", name="wy")
                    nc.vector.scalar_tensor_tensor(
                        wy[:], py2_t, gy2_c[:, c : c + 1], vy[:],
                        op0=Alu.min, op1=Alu.add)
                    wxr3 = wk.tile([128, P], F32, tag="vx", name="wxr3")
                    nc.scalar.activation(
                        wxr3[:], wx[:], Act.Relu, scale=3.0)
                    inter3 = wk.tile([128, P], F32, tag="vy", name="inter3")
                    nc.gpsimd.tensor_tensor(
                        inter3[:, 0:MSPLIT], wxr3[:, 0:MSPLIT],
                        wy[:, 0:MSPLIT], op=Alu.mult)
                    nc.vector.tensor_tensor(
                        inter3[:, MSPLIT:P], wxr3[:, MSPLIT:P],
                        wy[:, MSPLIT:P], op=Alu.mult)
                    pen = wk.tile([128, P], F32, tag="wx", name="pen")
                    nc.gpsimd.tensor_scalar(
                        pen[:], invp_t, gscal[:, 40 + c : 41 + c], None,
                        op0=Alu.mult)
                    nc.vector.tensor_tensor(
                        inter3[:], inter3[:], pen[:], op=Alu.subtract)
                    condv = wk.tile([128, P], F32, tag="vx", name="condv")
                    nc.vector.scalar_tensor_tensor(
                        condv[:], inter3[:], ag_c[:, c : c + 1], apeps_t,
                        op0=Alu.subtract, op1=Alu.is_gt,
                        accum_out=Scol[:, c : c + 1])
                    for k8 in range(P // 512):
                        nc.tensor.matmul(
                            nt[:, k8 * 512 : (k8 + 1) * 512], ones128[:],
                            condv[:, k8 * 512 : (k8 + 1) * 512],
                            start=(c == 0), stop=(c == NCH - 1))

                indg = sca.tile([128, NCH], F32, tag="indg", name="indg")
                nc.vector.tensor_scalar(indg[:], Scol[:], 0.0, None, op0=Alu.is_gt)
                nc.vector.tensor_reduce(
                    counts_sb[:, 12 + b : 13 + b], indg[:],
                    axis=mybir.AxisListType.X, op=Alu.add)
                nti = sca.tile([1, P], F32, tag="nti", name="nti")
                nc.scalar.activation(nti[:], nt[:], Act.Sign)
                nc.vector.tensor_reduce(
                    counts_sb[0:1, b : b + 1], nti[:],
                    axis=mybir.AxisListType.X, op=Alu.add)

            counts_ps = ps.tile([1, 16], F32, tag="nt", name="cps")
            nc.tensor.matmul(
                counts_ps[:], ones128[:], counts_sb[:], start=True, stop=True)
            counts_out = cst.tile([1, 16], F32)
            nc.vector.tensor_copy(counts_out[:], counts_ps[:])
            nc.sync.dma_start(counts_d[:], counts_out[:])

    nc.compile()
    return nc


def _sort_and_masks(pred, gt):
    """Sort preds/gts by area per batch (counts are permutation-invariant),
    compute per-core tile eligibility, and union it across the 8 cores
    (SPMD: one program for all cores). A tile is skippable iff every gt
    area is >= 2x every pred area in it, or vice versa."""
    ap = pred[..., 2] * pred[..., 3]   # [B, P]
    ag = gt[..., 2] * gt[..., 3]       # [B, G]
    pred_s = np.empty_like(pred)
    gt_dev = np.empty_like(gt)
    elig = np.zeros((N_CORES, BPC, NCH, NPG), dtype=bool)
    for b in range(B_TOTAL):
        po = np.argsort(ap[b], kind="stable")
        go = np.argsort(ag[b], kind="stable")
        pred_s[b] = pred[b][po]
        gts = gt[b][go]
        # device layout: partition p slot x holds gt box 8p+x; chunk c must
        # be the sorted-contiguous block [128c, 128c+128)
        gt_dev[b] = gts.reshape(NCH, 128, 6).transpose(1, 0, 2).reshape(G, 6)
        aps = ap[b][po]
        ags = ag[b][go]
        core, slot = b // BPC, b % BPC
        for c in range(NCH):
            glo, ghi = ags[c * 128], ags[c * 128 + 127]
            for pg in range(NPG):
                plo, phi = aps[pg * PG], aps[pg * PG + PG - 1]
                elig[core, slot, c, pg] = (glo < 2 * phi) and (ghi > plo / 2)
    union = elig.any(axis=0)  # [BPC, NCH, NPG]
    masks = tuple(tuple(tuple(bool(x) for x in row) for row in slot)
                  for slot in union)
    return pred_s, gt_dev, masks


def _get_program(with_mask: bool, masks=None):
    key = (with_mask, masks)
    if key not in _PROGRAM_CACHE:
        _PROGRAM_CACHE[key] = (
            _build_masked() if with_mask else _build_fast(masks)
        )
    return _PROGRAM_CACHE[key]


def _run_device(pred, gt, with_mask, masks=None, trace=False):
    nc = _get_program(with_mask, masks)
    in_maps = [
        {
            "pred": np.ascontiguousarray(pred[i * BPC : (i + 1) * BPC]),
            "gt": np.ascontiguousarray(gt[i * BPC : (i + 1) * BPC]),
        }
        for i in range(N_CORES)
    ]
    res = run_bass_kernel_spmd(nc, in_maps, list(range(N_CORES)), trace=trace)
    counts = np.stack([res.results[i]["counts"][0] for i in range(N_CORES)])
    return counts, res  # counts: [N_CORES, 16]


def kernel(pred_boxes, gt_boxes, _trace=False):
    pred = np.asarray(pred_boxes, dtype=np.float32)
    gt = np.asarray(gt_boxes, dtype=np.float32)
    assert pred.shape == (B_TOTAL, P, 6) and gt.shape == (B_TOTAL, G, 6)

    # the ignore mask only differs from all-ones when a pred AND a gt box are
    # both padding (cx == -1); specialize the program accordingly
    with_mask = bool((pred[..., 0] == -1.0).any() and (gt[..., 0] == -1.0).any())

    if with_mask:
        counts, res = _run_device(pred, gt, True, trace=_trace)
    else:
        pred_s, gt_dev, masks = _sort_and_masks(pred, gt)
        counts, res = _run_device(pred_s, gt_dev, False, masks, trace=_trace)
    kernel.last_results = res

    num_pos = counts[:, 0:4].reshape(-1).astype(np.float32)
    num_true = counts[:, 12:16].reshape(-1).astype(np.float32)
    if with_mask:
        num_pred = counts[:, 4:8].reshape(-1).astype(np.float32)
        num_gt = counts[:, 8:12].reshape(-1).astype(np.float32)
    else:
        # all boxes valid (host-verified): counts are the full box counts
        num_pred = np.full(B_TOTAL, np.float32(P), dtype=np.float32)
        num_gt = np.full(B_TOTAL, np.float32(G), dtype=np.float32)

    eps = np.float32(EPS)
    precision = num_pos / (num_pred + eps)
    recall = num_true / (num_gt + eps)
    fmeasure = np.float32(2.0) * (precision * recall) / (precision + recall + eps)
    return (precision, recall, fmeasure)
